# revision 9
# baseline (speedup 1.0000x reference)
"""DeepGraphSAGE (4x SAGEConv + BN/ReLU) on 8 Trainium2 NeuronCores.

Sharding: nodes partitioned across 8 cores (6250 dst nodes each).
Key structure vs the v1 kernel:
  - Each layer's AllGather is split into 4 quarter-chunks; edges are grouped
    by (dst block, src quarter) so gather+aggregate pipeline against the
    collective chunks as they arrive.
  - dma_gather uses prepare_only + trigger_dma on 4 SWDGE queues: descriptor
    generation (the serial gpsimd bottleneck) runs while collectives and
    earlier DMAs are in flight; the trigger carries the data dependency.
  - deginv is folded into the one-hot selection matrices (S values are
    deginv[dst] instead of 1.0), removing the normalize pass.
  - Aggregation accumulates across the 4 quarter groups in an SBUF f16
    buffer that is later overwritten by the pre-BN activations (aliased).
  - Layers 2/3/4 share one gather index/selection table (same edge order);
    gather indices are sorted ascending per chunk for HBM locality.
"""
import sys
import numpy as np

for p in ("/opt/trn_rl_repo",):
    if p not in sys.path:
        sys.path.append(p)

import concourse.bass as bass
import concourse.bacc as bacc
import concourse.mybir as mybir
from concourse.tile import TileContext
from concourse.masks import make_identity
from concourse.bass_utils import run_bass_kernel_spmd

f32 = mybir.dt.float32
f16 = mybir.dt.float16
i16 = mybir.dt.int16

NCORES = 8
P = 128
NQ = 4                 # SWDGE queues
USE_TRIGGER = True     # prepare_only + trigger_dma path
EPS = 1e-5
LAST_BUILD = None


# ---------------------------------------------------------------- host prep
class Grouping:
    """Edge grouping by (dst block, src quarter) for one table family.

    quarter_of(src) -> (j, table-relative row). Chunk counts are padded to
    the cross-core max so a single SPMD program fits every core; pad slots
    duplicate a valid index (S rows are zero there).
    """

    def __init__(self, n_own, nblk):
        self.n_own = n_own
        self.nblk = nblk
        self.kmax = None          # [(b,j)] -> padded chunk count
        self.calls = None         # list of (j, blocks, ktot, choff)

    @staticmethod
    def build(all_edges, n_own, nblk, quarter_fn, deginv_g):
        """all_edges: per-core list of (src_global, dst_local) arrays.
        quarter_fn(src)->(j, rel). Returns (grouping, per-core data dict)."""
        g = Grouping(n_own, nblk)
        ncores = len(all_edges)
        # per core, per (b, j): (rel_idx sorted, dst_in_block)
        percore = []
        for c in range(ncores):
            es, ed = all_edges[c]
            j_arr, rel = quarter_fn(es)
            blk = ed // P
            groups = {}
            for b in range(nblk):
                mb = blk == b
                for j in range(4):
                    m = mb & (j_arr == j)
                    r, dloc = rel[m], ed[m] - b * P
                    order = np.argsort(r, kind="stable")
                    groups[(b, j)] = (r[order], dloc[order])
            percore.append(groups)
        kmax = {}
        for b in range(nblk):
            for j in range(4):
                kmax[(b, j)] = max(
                    (len(percore[c][(b, j)][0]) + P - 1) // P for c in range(ncores)
                ) or 0
        g.kmax = kmax
        # calls: per (j, gtile of 4 blocks)
        calls = []
        choff = 0
        for j in range(4):
            for g0 in range(0, nblk, 4):
                blocks = list(range(g0, min(g0 + 4, nblk)))
                ktot = sum(kmax[(b, j)] for b in blocks)
                calls.append((j, blocks, ktot, choff))
                choff += ktot
        g.calls = calls
        g.totch = choff
        return g, percore

    def build_streams(self, percore_c, core, deginv_core):
        """idx stream [128, totch*8] i16 and S [128, totch, 128] f16 for one
        core. deginv_core: deginv for this core's own dst nodes [n_own]."""
        totch = self.totch
        iv = np.zeros(totch * P, np.int16)
        S = np.zeros((P, totch, P), np.float16)
        for (j, blocks, ktot, choff) in self.calls:
            off = choff
            for b in blocks:
                r, dloc = percore_c[(b, j)]
                k = self.kmax[(b, j)]
                n = len(r)
                for ci in range(k):
                    s0 = ci * P
                    v = r[s0:s0 + P]
                    base = off + ci
                    if len(v):
                        iv[base * P:base * P + len(v)] = v
                        if len(v) < P:  # pad with first idx of chunk
                            iv[base * P + len(v):(base + 1) * P] = v[0]
                        dl = dloc[s0:s0 + P]
                        S[np.arange(len(dl)), base, dl] = deginv_core[
                            b * P + dl].astype(np.float16)
                    else:  # all-pad chunk: gather row 0, S row zero
                        iv[base * P:(base + 1) * P] = 0
                off += k
        w = iv.reshape(-1, 16).T                    # [16, totch*8]
        idx16 = np.tile(w, (8, 1)).copy()           # [128, totch*8]
        return idx16, S


# ---------------------------------------------------------------- program
def build_program(n_nodes, in_f, hid, out_f, gA, gB, qlen):
    nown = n_nodes // NCORES
    nblk = (nown + P - 1) // P
    pad_n = nblk * P
    ntile = (nown + 511) // 512
    nfc = hid // P
    qoff = np.concatenate([[0], np.cumsum(qlen)])

    nc = bacc.Bacc("TRN2", target_bir_lowering=False, debug=False,
                   num_devices=NCORES, num_swdge_queues=NQ)

    # ---- I/O ----
    x16 = nc.dram_tensor("x16", [n_nodes, 128], f16, kind="ExternalInput")
    xT = nc.dram_tensor("xT", [in_f, pad_n], f16, kind="ExternalInput")
    idxA_d = nc.dram_tensor("idxA", [P, max(gA.totch * 8, 8)], i16, kind="ExternalInput")
    sA_d = nc.dram_tensor("sA", [P, max(gA.totch, 1), P], f16, kind="ExternalInput")
    idxB_d = nc.dram_tensor("idxB", [P, max(gB.totch * 8, 8)], i16, kind="ExternalInput")
    sB_d = nc.dram_tensor("sB", [P, max(gB.totch, 1), P], f16, kind="ExternalInput")
    wl_d, wr_d, g_d, b_d = {}, {}, {}, {}
    dims = [(in_f, hid), (hid, hid), (hid, hid), (hid, out_f)]
    for l, (fi, fo) in enumerate(dims, start=1):
        wl_d[l] = nc.dram_tensor(f"Wl{l}", [fi, fo], f16, kind="ExternalInput")
        wr_d[l] = nc.dram_tensor(f"Wr{l}", [fi, fo], f16, kind="ExternalInput")
    for l in (1, 2, 3):
        g_d[l] = nc.dram_tensor(f"g{l}", [hid], f32, kind="ExternalInput")
        b_d[l] = nc.dram_tensor(f"b{l}", [hid], f32, kind="ExternalInput")
    bl4_d = nc.dram_tensor("bl4", [out_f], f32, kind="ExternalInput")
    out_d = nc.dram_tensor("out", [nown, out_f], f32, kind="ExternalOutput")

    # ---- internal DRAM ----
    h_own = {j: nc.dram_tensor(f"hq{j}_own", [qlen[j], hid], f16) for j in range(4)}
    h_all = {j: nc.dram_tensor(f"hq{j}_all", [NCORES * qlen[j], hid], f16,
                               addr_space="Shared") for j in range(4)}
    y_own = {j: nc.dram_tensor(f"yq{j}_own", [qlen[j], 128], f16) for j in range(4)}
    y_all = {j: nc.dram_tensor(f"yq{j}_all", [NCORES * qlen[j], 128], f16,
                               addr_space="Shared") for j in range(4)}
    st_in = {l: nc.dram_tensor(f"st{l}_in", [P, 8], f32) for l in (1, 2, 3)}
    st_out = {l: nc.dram_tensor(f"st{l}_out", [P, 8], f32, addr_space="Shared")
              for l in (1, 2, 3)}
    rg = [list(range(NCORES))]

    dma_sems = None

    with TileContext(nc) as tc:
        with (
            tc.tile_pool(name="const", bufs=1) as cp,
            tc.tile_pool(name="gbuf", bufs=2) as gp,
            tc.tile_pool(name="sbuf", bufs=3) as sp,
            tc.tile_pool(name="small", bufs=3) as sm,
            tc.tile_pool(name="aggt", bufs=2) as at,
            tc.tile_pool(name="psA", bufs=3, space="PSUM") as psA,
            tc.tile_pool(name="psB", bufs=2, space="PSUM") as psB,
            tc.tile_pool(name="psC", bufs=2, space="PSUM") as psC,
        ):
            dma_sems = [nc.alloc_semaphore(f"swdge_dma{q}") for q in range(NQ)]
            ident = cp.tile([P, P], f16)
            make_identity(nc, ident[:])
            ident32 = cp.tile([P, P], f32)
            make_identity(nc, ident32[:])
            # weights resident in SBUF
            W = {}
            for l, (fi, fo) in enumerate(dims, start=1):
                kc = (fi + P - 1) // P
                for (nm, dram) in (("l", wl_d[l]), ("r", wr_d[l])):
                    for q in range(kc):
                        r0, r1 = q * P, min((q + 1) * P, fi)
                        t = cp.tile([r1 - r0, fo], f16, tag=f"W{nm}{l}_{q}")
                        nc.sync.dma_start(out=t[:], in_=dram[r0:r1, :])
                        W[(nm, l, q)] = t
            gb = {}
            for l in (1, 2, 3):
                for nm, dram in (("g", g_d[l]), ("b", b_d[l])):
                    t = cp.tile([P, nfc], f32, tag=f"{nm}{l}")
                    nc.sync.dma_start(out=t[:], in_=dram[:].rearrange("(c p) -> p c", p=P))
                    gb[(nm, l)] = t
            bl4_t = cp.tile([P, 1], f32)
            nc.sync.dma_start(out=bl4_t[:out_f, :], in_=bl4_d[:, None])
            # resident gather-index table for layers 2/3/4
            idxB_t = cp.tile([P, max(gB.totch * 8, 8)], i16)
            nc.sync.dma_start(out=idxB_t[:], in_=idxB_d[:, :])

            # acc/preBN buffer (aliased) and hT buffer, tile-major layout:
            # bufA col b*512 == (tile t=b//4) region; preBN(q,t) at t*2048+q*512
            bufA = cp.tile([P, ntile * nfc * 512], f16, name="bufA")
            bufB = cp.tile([P, ntile * nfc * 512], f16, name="bufB")

            qcnt = [0]  # round-robin over gather queues

            def acc_slice(b, width):
                return bufA[:, b * 512:b * 512 + width]

            def preBN_slice(q, t, n0, n1):
                base = t * 2048 + q * 512
                return bufA[:, base + n0:base + n1]

            def hT_slice(q, t, n0, n1):
                base = t * 2048 + q * 512
                return bufB[:, base + n0:base + n1]

            def aggregate(grouping, idx_tile, idx_dram, s_dram, tables, width,
                          row_elems, use_trigger):
                """Gather+aggregate all 4 quarter groups into bufA (f16).

                Triggers are emitted one call behind the preps so descriptor
                generation of call N+1 overlaps call N's AG wait / DMA drain.
                """
                first_seen = set()
                pending_trig = []  # queue nums awaiting trigger emission

                def consume(j, blocks, ktot, choff, S, G):
                    off = choff
                    for b in blocks:
                        k = grouping.kmax[(b, j)]
                        if k == 0:
                            continue
                        ps = psA.tile([P, 512], f32, tag="aggps")
                        for ci in range(k):
                            cj = off - choff + ci
                            nc.tensor.matmul(
                                out=ps[:, :width],
                                lhsT=S[:, cj, :], rhs=G[:, cj, :width],
                                start=(ci == 0), stop=(ci == k - 1),
                            )
                        dstap = acc_slice(b, width)
                        if b not in first_seen:
                            first_seen.add(b)
                            nc.vector.tensor_copy(out=dstap[:], in_=ps[:, :width])
                        else:
                            nc.vector.tensor_tensor(
                                out=dstap[:], in0=dstap[:],
                                in1=ps[:, :width], op=mybir.AluOpType.add)
                        off += k

                pending_consume = []
                for (j, blocks, ktot, choff) in grouping.calls:
                    if ktot == 0:
                        continue
                    q = qcnt[0] % NQ
                    qcnt[0] += 1
                    G = gp.tile([P, ktot, row_elems], f16, tag=f"G{row_elems}")
                    S = sp.tile([P, ktot, P], f16, tag="S")
                    nc.scalar.dma_start(out=S[:], in_=s_dram[:, choff:choff + ktot, :])
                    if idx_tile is not None:
                        idx_ap = idx_tile[:, choff * 8:(choff + ktot) * 8]
                    else:
                        it = sm.tile([P, ktot * 8], i16, tag="idxs")
                        nc.scalar.dma_start(out=it[:],
                                            in_=idx_dram[:, choff * 8:(choff + ktot) * 8])
                        idx_ap = it[:]
                    if use_trigger:
                        nc.gpsimd.dma_gather(
                            out_ap=G[:], in_ap=tables[j],
                            idxs_ap=idx_ap,
                            num_idxs=ktot * P, num_idxs_reg=ktot * P,
                            elem_size=row_elems, single_packet=False,
                            prepare_only=True, sem=dma_sems[q], queue_num=q,
                        )
                        # fire the previous call's DMA; its consume follows
                        if pending_trig:
                            pq = pending_trig.pop(0)
                            nc.gpsimd.trigger_dma(count=None, queue_num=pq)
                            consume(*pending_consume.pop(0))
                        pending_trig.append(q)
                        pending_consume.append((j, blocks, ktot, choff, S, G))
                    else:
                        nc.gpsimd.dma_gather(
                            out_ap=G[:], in_ap=tables[j],
                            idxs_ap=idx_ap,
                            num_idxs=ktot * P, num_idxs_reg=ktot * P,
                            elem_size=row_elems, single_packet=False,
                            queue_num=q,
                        )
                        consume(j, blocks, ktot, choff, S, G)
                while pending_trig:
                    pq = pending_trig.pop(0)
                    nc.gpsimd.trigger_dma(count=None, queue_num=pq)
                    consume(*pending_consume.pop(0))

            def make_aggT(t, fi_chunks, width):
                """Transpose acc blocks of tile t into aggT tiles [fi, 512]."""
                blocks = range(4 * t, min(4 * t + 4, nblk))
                aggT = [at.tile([P, 512], f16, tag=f"aggT{q}", name=f"aggT{q}_{t}")
                        for q in range(fi_chunks)]
                for bi, b in enumerate(blocks):
                    tp = psB.tile([P, 512], f16, tag="tp")
                    for q in range(fi_chunks):
                        w0 = q * P
                        w1 = min(w0 + P, width)
                        if w0 >= width:
                            break
                        nc.tensor.matmul(out=tp[:w1 - w0, q * P:q * P + P],
                                         lhsT=acc_slice(b, 512)[:, w0:w1],
                                         rhs=ident[:], is_transpose=True)
                        nc.vector.tensor_copy(
                            out=aggT[q][:w1 - w0, bi * P:(bi + 1) * P],
                            in_=tp[:w1 - w0, q * P:q * P + P])
                return aggT

            def dense_and_bn(l, fi_chunks, width, rhs_root):
                """Dense (agg@Wl + root@Wr) -> stats -> preBN (into bufA)."""
                stats = [sm.tile([P, ntile * 6], f32, tag=f"stats{q}",
                                 name=f"stats{l}_{q}") for q in range(nfc)]
                for t in range(ntile):
                    ns, ne = t * 512, min((t + 1) * 512, nown)
                    nn = ne - ns
                    aggT = make_aggT(t, fi_chunks, width)
                    roots = rhs_root(t, ns, ne)
                    for fo in range(nfc):
                        dps = psC.tile([P, 512], f32, tag="dense")
                        nmm = 2 * fi_chunks
                        mm = 0
                        for q in range(fi_chunks):
                            w0 = q * P
                            w1 = min(w0 + P, width)
                            nc.tensor.matmul(out=dps[:, :nn],
                                             lhsT=W[("l", l, q)][:, fo * P:(fo + 1) * P],
                                             rhs=aggT[q][:w1 - w0, :nn],
                                             start=(mm == 0), stop=(mm == nmm - 1))
                            mm += 1
                            nc.tensor.matmul(out=dps[:, :nn],
                                             lhsT=W[("r", l, q)][:, fo * P:(fo + 1) * P],
                                             rhs=roots[q],
                                             start=False, stop=(mm == nmm - 1))
                            mm += 1
                        nc.vector.bn_stats(out=stats[fo][:, t * 6:(t + 1) * 6],
                                           in_=dps[:, :nn])
                        nc.vector.tensor_copy(out=preBN_slice(fo, t, 0, nn),
                                              in_=dps[:, :nn])
                return stats

            def bn_allreduce(l, stats):
                pack = sm.tile([P, 8], f32, tag="pack")
                for q in range(nfc):
                    mv = sm.tile([P, 2], f32, tag="mv")
                    nc.vector.bn_aggr(out=mv[:], in_=stats[q][:])
                    sq = sm.tile([P, 1], f32, tag="sq")
                    nc.vector.tensor_tensor(out=sq[:], in0=mv[:, 0:1],
                                            in1=mv[:, 0:1], op=mybir.AluOpType.mult)
                    nc.vector.tensor_tensor(out=sq[:], in0=sq[:], in1=mv[:, 1:2],
                                            op=mybir.AluOpType.add)
                    nc.vector.tensor_scalar(out=pack[:, 2 * q:2 * q + 1],
                                            in0=mv[:, 0:1], scalar1=float(nown),
                                            scalar2=None, op0=mybir.AluOpType.mult)
                    nc.vector.tensor_scalar(out=pack[:, 2 * q + 1:2 * q + 2],
                                            in0=sq[:], scalar1=float(nown),
                                            scalar2=None, op0=mybir.AluOpType.mult)
                nc.sync.dma_start(out=st_in[l][:, :], in_=pack[:])
                nc.gpsimd.collective_compute(
                    "AllReduce", mybir.AluOpType.add, replica_groups=rg,
                    ins=[st_in[l][:, :]], outs=[st_out[l][:, :]],
                )
                red = sm.tile([P, 8], f32, tag="red")
                nc.sync.dma_start(out=red[:], in_=st_out[l][:, :])
                scale = sm.tile([P, nfc], f32, tag="scale", name=f"scale{l}")
                shift = sm.tile([P, nfc], f32, tag="shift", name=f"shift{l}")
                inv_n = 1.0 / float(n_nodes)
                for q in range(nfc):
                    mu = sm.tile([P, 1], f32, tag="mu")
                    var = sm.tile([P, 1], f32, tag="var")
                    nc.vector.tensor_scalar(out=mu[:], in0=red[:, 2 * q:2 * q + 1],
                                            scalar1=inv_n, scalar2=None,
                                            op0=mybir.AluOpType.mult)
                    nc.vector.tensor_scalar(out=var[:], in0=red[:, 2 * q + 1:2 * q + 2],
                                            scalar1=inv_n, scalar2=None,
                                            op0=mybir.AluOpType.mult)
                    musq = sm.tile([P, 1], f32, tag="musq")
                    nc.vector.tensor_tensor(out=musq[:], in0=mu[:], in1=mu[:],
                                            op=mybir.AluOpType.mult)
                    nc.vector.tensor_tensor(out=var[:], in0=var[:], in1=musq[:],
                                            op=mybir.AluOpType.subtract)
                    nc.vector.tensor_scalar(out=var[:], in0=var[:], scalar1=EPS,
                                            scalar2=None, op0=mybir.AluOpType.add)
                    nc.vector.reciprocal(out=var[:], in_=var[:])
                    rs = sm.tile([P, 1], f32, tag="rs")
                    nc.scalar.activation(out=rs[:], in_=var[:],
                                         func=mybir.ActivationFunctionType.Sqrt)
                    nc.vector.tensor_tensor(out=scale[:, q:q + 1], in0=rs[:],
                                            in1=gb[("g", l)][:, q:q + 1],
                                            op=mybir.AluOpType.mult)
                    nc.vector.tensor_tensor(out=musq[:], in0=mu[:],
                                            in1=scale[:, q:q + 1],
                                            op=mybir.AluOpType.mult)
                    nc.vector.tensor_tensor(out=shift[:, q:q + 1],
                                            in0=gb[("b", l)][:, q:q + 1], in1=musq[:],
                                            op=mybir.AluOpType.subtract)
                return scale, shift

            def bn_rows_ag(l, scale, shift):
                """BN+ReLU preBN->hT; transpose to rows; DMA; quarter AGs."""
                for t in range(ntile):
                    ns, ne = t * 512, min((t + 1) * 512, nown)
                    nn = ne - ns
                    for q in range(nfc):
                        nc.scalar.activation(
                            out=hT_slice(q, t, 0, nn), in_=preBN_slice(q, t, 0, nn),
                            func=mybir.ActivationFunctionType.Relu,
                            bias=shift[:, q:q + 1], scale=scale[:, q:q + 1],
                        )
                # rows per block, grouped by quarter; AG fires per quarter
                b0 = 0
                for j in range(4):
                    rows_in_q = qlen[j]
                    nb = (rows_in_q + P - 1) // P
                    for bi in range(nb):
                        b = b0 + bi
                        t, i = b // 4, b % 4
                        r0 = bi * P
                        r1 = min(r0 + P, rows_in_q)
                        tpr = psB.tile([P, 512], f16, tag="tp")
                        for q in range(nfc):
                            nc.tensor.matmul(
                                out=tpr[:, q * P:(q + 1) * P],
                                lhsT=hT_slice(q, t, i * P, i * P + P)[:, :],
                                rhs=ident[:], is_transpose=True)
                        rows = sm.tile([P, hid], f16, tag="rows")
                        nc.vector.tensor_copy(out=rows[:], in_=tpr[:, :hid])
                        nc.sync.dma_start(out=h_own[j][r0:r1, :],
                                          in_=rows[:r1 - r0, :])
                    b0 += nb
                    nc.gpsimd.collective_compute(
                        "AllGather", mybir.AluOpType.bypass, replica_groups=rg,
                        ins=[h_own[j][:, :]], outs=[h_all[j][:, :]],
                    )

            # ================= layer 1 =================
            x_tables = [x16[12500 * j:12500 * (j + 1), :] for j in range(4)]
            aggregate(gA, None, idxA_d, sA_d, x_tables, in_f, 128,
                      use_trigger=USE_TRIGGER)

            def xT_root(t, ns, ne):
                xt = sm.tile([in_f, 512], f16, tag="xTt")
                nc.sync.dma_start(out=xt[:, :ne - ns], in_=xT[:, ns:ne])
                return [xt[:, :ne - ns]]

            stats = dense_and_bn(1, 1, in_f, xT_root)
            scale, shift = bn_allreduce(1, stats)
            bn_rows_ag(1, scale, shift)

            # ================= layers 2,3 =================
            for l in (2, 3):
                h_tables = [h_all[j][:, :] for j in range(4)]
                aggregate(gB, idxB_t, None, sB_d, h_tables, hid, hid,
                          use_trigger=USE_TRIGGER)

                def h_root(t, ns, ne, _l=l):
                    return [hT_slice(q, t, 0, ne - ns) for q in range(nfc)]

                stats = dense_and_bn(l, nfc, hid, h_root)
                scale, shift = bn_allreduce(l, stats)
                bn_rows_ag(l, scale, shift)

            # ================= layer 4 =================
            # y = h3 @ Wl4 (transposed), to rows, quarter AGs
            for t in range(ntile):
                ns, ne = t * 512, min((t + 1) * 512, nown)
                nn = ne - ns
                yps = psC.tile([P, 512], f32, tag="dense")
                for q in range(nfc):
                    nc.tensor.matmul(out=yps[:out_f, :nn],
                                     lhsT=W[("l", 4, q)][:, :out_f],
                                     rhs=hT_slice(q, t, 0, nn),
                                     start=(q == 0), stop=(q == nfc - 1))
                ysb = sm.tile([P, 512], f16, tag="ysb")
                nc.vector.tensor_copy(out=ysb[:out_f, :nn], in_=yps[:out_f, :nn])
                for bi in range((nn + P - 1) // P):
                    c0 = bi * P
                    c1 = min(c0 + P, nn)
                    tpy = psB.tile([P, 512], f16, tag="tp")
                    nc.tensor.matmul(out=tpy[:c1 - c0, :out_f],
                                     lhsT=ysb[:out_f, c0:c1],
                                     rhs=ident[:out_f, :out_f],
                                     is_transpose=True)
                    yr = sm.tile([P, 128], f16, tag="yrows")
                    nc.vector.memset(yr[:], 0.0)
                    nc.vector.tensor_copy(out=yr[:c1 - c0, :out_f],
                                          in_=tpy[:c1 - c0, :out_f])
                    b = 4 * t + bi
                    # global row b*128+... falls in quarter j at offset r0
                    gr0 = b * P
                    j = int(np.searchsorted(qoff, gr0, side="right") - 1)
                    r0 = gr0 - qoff[j]
                    nc.sync.dma_start(out=y_own[j][r0:r0 + (c1 - c0), :],
                                      in_=yr[:c1 - c0, :])
            for j in range(4):
                nc.gpsimd.collective_compute(
                    "AllGather", mybir.AluOpType.bypass, replica_groups=rg,
                    ins=[y_own[j][:, :]], outs=[y_all[j][:, :]],
                )
            # aggregate y (quartered), reusing the B grouping/streams
            y_tables = [y_all[j][:, :] for j in range(4)]
            aggregate(gB, idxB_t, None, sB_d, y_tables, out_f, 128,
                      use_trigger=USE_TRIGGER)
            # final: out = agg4 + h3 @ Wr4 + bl4
            for t in range(ntile):
                ns, ne = t * 512, min((t + 1) * 512, nown)
                nn = ne - ns
                aggT = make_aggT(t, 1, out_f)
                ops = psC.tile([P, 512], f32, tag="dense")
                for q in range(nfc):
                    nc.tensor.matmul(out=ops[:out_f, :nn],
                                     lhsT=W[("r", 4, q)][:, :out_f],
                                     rhs=hT_slice(q, t, 0, nn),
                                     start=(q == 0), stop=(q == nfc - 1))
                osb = sm.tile([P, 512], f32, tag="osb")
                nc.vector.tensor_tensor(out=osb[:out_f, :nn], in0=ops[:out_f, :nn],
                                        in1=aggT[0][:out_f, :nn],
                                        op=mybir.AluOpType.add)
                nc.vector.tensor_scalar(out=osb[:out_f, :nn], in0=osb[:out_f, :nn],
                                        scalar1=bl4_t[:out_f, 0:1], scalar2=None,
                                        op0=mybir.AluOpType.add)
                for bi in range((nn + P - 1) // P):
                    c0, c1 = bi * P, min(bi * P + P, nn)
                    tpo = psB.tile([P, 512], f32, tag="tp")
                    nc.tensor.matmul(out=tpo[:c1 - c0, :out_f],
                                     lhsT=osb[:out_f, c0:c1],
                                     rhs=ident32[:out_f, :out_f],
                                     is_transpose=True)
                    orow = sm.tile([P, out_f], f32, tag="orow")
                    nc.vector.tensor_copy(out=orow[:c1 - c0, :],
                                          in_=tpo[:c1 - c0, :out_f])
                    nc.sync.dma_start(out=out_d[ns + c0:ns + c1, :],
                                      in_=orow[:c1 - c0, :])
    return nc


def kernel(**inputs):
    x = np.asarray(inputs["x"], np.float32)
    edge_index = np.asarray(inputs["edge_index"])
    n_nodes, in_f = x.shape
    hid = inputs["Wl2"].shape[0]
    out_f = inputs["Wl4"].shape[1]
    nown = n_nodes // NCORES
    nblk = (nown + P - 1) // P

    src = np.asarray(edge_index[0]).astype(np.int64)
    dst = np.asarray(edge_index[1]).astype(np.int64)
    deg = np.bincount(dst, minlength=n_nodes).astype(np.float32)
    deginv = (1.0 / np.maximum(deg, 1.0)).astype(np.float32)

    # quarter lengths of the per-core row space (block-aligned except last)
    q_blocks = [13, 13, 13, nblk - 39]
    qlen = []
    off = 0
    for nbq in q_blocks:
        ln = min(nbq * P, nown - off)
        qlen.append(ln)
        off += ln
    qoff = np.concatenate([[0], np.cumsum(qlen)])  # [0,1664,3328,4992,6250]

    # per-core edge lists (dst-sharded)
    all_edges = []
    for c in range(NCORES):
        lo = c * nown
        m = (dst >= lo) & (dst < lo + nown)
        all_edges.append((src[m], dst[m] - lo))

    # grouping A: src quartered by global node range (for x table)
    def quarter_global(s):
        j = np.minimum(s // 12500, 3).astype(np.int64)
        return j, (s - j * 12500).astype(np.int64)

    # grouping B: src quartered by (owner core, local quarter)
    def quarter_local(s):
        c = s // nown
        r = s - c * nown
        j = np.searchsorted(qoff, r, side="right") - 1
        rel = c * np.array(qlen)[j] + (r - qoff[j])
        return j.astype(np.int64), rel.astype(np.int64)

    gA, pcA = Grouping.build(all_edges, nown, nblk, quarter_global, deginv)
    gB, pcB = Grouping.build(all_edges, nown, nblk, quarter_local, deginv)

    import time as _time
    _t0 = _time.perf_counter()
    nc = build_program(n_nodes, in_f, hid, out_f, gA, gB, qlen)
    print(f"[kernel] program built in {_time.perf_counter() - _t0:.1f}s", flush=True)
    _t0 = _time.perf_counter()
    nc.compile()
    print(f"[kernel] bacc compile in {_time.perf_counter() - _t0:.1f}s", flush=True)

    x16 = np.zeros((n_nodes, 128), np.float16)
    x16[:, :in_f] = x.astype(np.float16)
    pad_n = nblk * P

    in_maps = []
    for c in range(NCORES):
        dgc = deginv[c * nown:(c + 1) * nown]
        idxA, sA = gA.build_streams(pcA[c], c, dgc)
        idxB, sB = gB.build_streams(pcB[c], c, dgc)
        xTc = np.zeros((in_f, pad_n), np.float16)
        xTc[:, :nown] = x[c * nown:(c + 1) * nown].T.astype(np.float16)
        im = {
            "x16": x16, "xT": xTc,
            "idxA": idxA if idxA.size else np.zeros((P, 8), np.int16),
            "sA": sA if sA.size else np.zeros((P, 1, P), np.float16),
            "idxB": idxB if idxB.size else np.zeros((P, 8), np.int16),
            "sB": sB if sB.size else np.zeros((P, 1, P), np.float16),
            "bl4": np.asarray(inputs["bl4"], np.float32),
        }
        for l in (1, 2, 3, 4):
            im[f"Wl{l}"] = np.asarray(inputs[f"Wl{l}"], np.float16)
            im[f"Wr{l}"] = np.asarray(inputs[f"Wr{l}"], np.float16)
        for l in (1, 2, 3):
            im[f"g{l}"] = np.asarray(inputs[f"g{l}"], np.float32)
            im[f"b{l}"] = np.asarray(inputs[f"b{l}"], np.float32)
        in_maps.append(im)

    global LAST_BUILD
    LAST_BUILD = (nc, in_maps)
    res = run_bass_kernel_spmd(nc, in_maps, list(range(NCORES)))
    out = np.concatenate([res.results[c]["out"] for c in range(NCORES)], axis=0)
    return out.astype(np.float32)
